# revision 1
# baseline (speedup 1.0000x reference)
"""CoconBlock forward on 8 Trainium2 NeuronCores.

Sharding: core c = (b, g) with b = c // 4 (batch), g = c % 4 (tensor-parallel
rank). Within each batch group of 4 cores:
  - attention QKV / context-KV weights column-sharded by head group (3 heads),
  - W_proj row-sharded, partial outputs AllReduced,
  - MLP W_fc column-sharded / W_mproj row-sharded, partial outputs (with the
    residual and bias pre-folded as +h2/4 + b_mproj/4 per core) ReduceScattered
    so each core lands exactly its 192-feature slice of the final output.

All on-device activations are feature-on-partition (f32 has no DMA transpose;
this layout makes every matmul transpose-free). LayerNorm reductions over the
feature (partition) axis run on the PE via ones-vector matmuls; the affine
(gamma/beta) is folded into the following weight matrix on the host. Engine
balance: exp/gelu/sqrt on ACT, every psum->sbuf copy/bias-add on DVE.

Attention uses the 128-aligned causal structure: with queries padded to 1152
and keys (256 context + 1025 self) padded to 1408, a (query-block, key-tile)
pair is fully allowed, partially masked by a shifted-triangle slice of one
master mask, or skipped entirely. Scores for a 512-query block are exp'ed in
one ACT op per key tile; the softmax denominator rides the attend matmul as a
ones-column appended to V (partition 64), and 1/den is broadcast to partitions
0..63 with a K=1 PE outer product.
"""

import sys

sys.path.insert(0, "/opt/trn_rl_repo")

import ml_dtypes
import numpy as np

import concourse.bass as bass
import concourse.bacc as bacc
import concourse.mybir as mybir
import concourse.tile as tile
from concourse.bass_utils import run_bass_kernel_spmd

F32 = mybir.dt.float32
AF = mybir.ActivationFunctionType
ALU = mybir.AluOpType
ts, ds = bass.ts, bass.ds

D = 768
DH = 64
S = 1024
SC = 256
TOK = S + 1            # 1025 (sos + x)
TOKP = 1152            # 9 * 128
NT = TOKP // 128       # 9
KEYSP = 1408           # 11 * 128
NK = KEYSP // 128      # 11
KO = D // 128          # 6 feature sub-tiles
TP = 4
FG = 192               # features per core in head-sharded tensors (3 heads)
HG = 3                 # heads per core
EPS = 1e-5
N_CORES = 8
GROUPS = [[0, 1, 2, 3], [4, 5, 6, 7]]
BLKS = [(0, 512), (512, 512), (1024, 128)]  # token blocks (start, len)


BF16 = mybir.dt.bfloat16


def _emit_ln(nc, pers, trans, psum, src, dst, onec_sb, oner_sb, eps_sb):
    """dst = (src - mean) * rsqrt(var + eps), stats over the feature axis."""
    m_row = trans.tile([1, TOKP], F32, tag="m_row", bufs=2)
    s_row = trans.tile([1, TOKP], F32, tag="s_row", bufs=2)
    for bs, bl in BLKS:
        sl = ds(bs, bl)
        ps_m = psum.tile([128, 512], F32, tag="sm", bufs=2)
        ps_s = psum.tile([128, 512], F32, tag="sm", bufs=2)
        for o in range(KO):
            sq = trans.tile([128, 512], BF16, tag="sq", bufs=2)
            nc.vector.tensor_mul(sq[:, :bl], src[:, o, sl], src[:, o, sl])
            nc.tensor.matmul(
                ps_m[0:1, :bl], onec_sb, src[:, o, sl],
                start=(o == 0), stop=(o == KO - 1),
            )
            nc.tensor.matmul(
                ps_s[0:1, :bl], onec_sb, sq[:, :bl],
                start=(o == 0), stop=(o == KO - 1),
            )
        nc.vector.tensor_scalar_mul(m_row[:, sl], ps_m[0:1, :bl], 1.0 / D)
        nc.vector.tensor_scalar_mul(s_row[:, sl], ps_s[0:1, :bl], 1.0 / D)
    # var = E[x^2] - mean^2 ; rstd = 1/sqrt(var + eps) ; mr = mean * rstd
    m2 = trans.tile([1, TOKP], F32, tag="lntmp", bufs=2)
    nc.vector.tensor_mul(m2[:], m_row[:], m_row[:])
    nc.vector.tensor_tensor(s_row[:], s_row[:], m2[:], ALU.subtract)
    nc.scalar.activation(s_row[:], s_row[:], AF.Sqrt, bias=eps_sb)
    nc.vector.reciprocal(s_row[:], s_row[:])
    nc.vector.tensor_mul(m_row[:], m_row[:], s_row[:])

    # broadcast rstd and mean*rstd across partitions (bf16 outer products;
    # bf16 rounding of rstd only scales the normalized values by ~0.4%)
    rs16 = trans.tile([1, TOKP], BF16, tag="rs16", bufs=2)
    mr16 = trans.tile([1, TOKP], BF16, tag="mr16", bufs=2)
    nc.vector.tensor_copy(rs16[:], s_row[:])
    nc.vector.tensor_copy(mr16[:], m_row[:])
    for bs, bl in BLKS:
        sl = ds(bs, bl)
        ps_b = psum.tile([128, 512], F32, tag="sm", bufs=2)
        nc.tensor.matmul(ps_b[:, :bl], oner_sb[0:1, 0:128], rs16[:, sl],
                         start=True, stop=True)
        for o in range(KO):
            # dst = src * rstd only; the -mean*rstd term is folded into the
            # consuming matmuls as a rank-1 correction (colsum(W) x mr)
            nc.vector.tensor_mul(dst[:, o, sl], src[:, o, sl], ps_b[:, :bl])
    return mr16


def build_program(sim_collectives=True, gelu_fn=None, debug_taps=False):
    if gelu_fn is None:
        gelu_fn = AF.Gelu_apprx_tanh
    nc = bacc.Bacc(None, num_devices=N_CORES)

    # ---- DRAM I/O ----
    hT_d = nc.dram_tensor("hT", [D, TOKP], BF16, kind="ExternalInput")
    ctxT_d = nc.dram_tensor("ctxT", [D, SC], BF16, kind="ExternalInput")
    wq_d = nc.dram_tensor("wq", [D, FG], BF16, kind="ExternalInput")
    wk_d = nc.dram_tensor("wk", [D, FG], BF16, kind="ExternalInput")
    wv_d = nc.dram_tensor("wv", [D, FG], BF16, kind="ExternalInput")
    wrk_d = nc.dram_tensor("wrk", [D, FG], BF16, kind="ExternalInput")
    wrv_d = nc.dram_tensor("wrv", [D, FG], BF16, kind="ExternalInput")
    bq_d = nc.dram_tensor("bq", [FG, 1], F32, kind="ExternalInput")
    bk_d = nc.dram_tensor("bk", [FG, 1], F32, kind="ExternalInput")
    brk_d = nc.dram_tensor("brk", [FG, 1], F32, kind="ExternalInput")
    bv_d = nc.dram_tensor("bv_r", [1, FG], BF16, kind="ExternalInput")
    brv_d = nc.dram_tensor("brv_r", [1, FG], BF16, kind="ExternalInput")
    wp0_d = nc.dram_tensor("wp0", [DH, D], BF16, kind="ExternalInput")
    wp1_d = nc.dram_tensor("wp1", [DH, D], BF16, kind="ExternalInput")
    wp2_d = nc.dram_tensor("wp2", [DH, D], BF16, kind="ExternalInput")
    bp4_d = nc.dram_tensor("bp4", [D, 1], F32, kind="ExternalInput")
    wfc_d = nc.dram_tensor("wfc", [D, D], BF16, kind="ExternalInput")
    bfc_d = nc.dram_tensor("bfc", [D, 1], F32, kind="ExternalInput")
    wmp_d = nc.dram_tensor("wmp", [D, D], BF16, kind="ExternalInput")
    bmp16_d = nc.dram_tensor("bmp16_r", [1, D], BF16, kind="ExternalInput")
    mask_d = nc.dram_tensor("mask", [128, 896], BF16, kind="ExternalInput")
    onec_d = nc.dram_tensor("onec", [128, 1], BF16, kind="ExternalInput")
    oner_d = nc.dram_tensor("oner", [1, 512], F32)
    crow_d = nc.dram_tensor("crow", [1, 1344], BF16, kind="ExternalInput")
    oner16_d = nc.dram_tensor("oner16", [1, 512], BF16, kind="ExternalInput")
    out_d = nc.dram_tensor("out", [FG, TOKP], BF16, kind="ExternalOutput")

    with tile.TileContext(nc) as tc, \
         tc.tile_pool(name="pers", bufs=1) as pers, \
         tc.tile_pool(name="trans", bufs=2) as trans, \
         tc.tile_pool(name="psum", bufs=1, space="PSUM") as psum, \
         tc.tile_pool(name="dram", bufs=1, space="DRAM") as dram:

        # ---- resident SBUF tensors ----
        h_main = pers.tile([128, KO, TOKP], BF16, tag="h_main")
        ctx = pers.tile([128, KO, SC], BF16, tag="ctx")
        wp_sb = [pers.tile([DH, D], BF16, tag=f"wp{h}", name=f"wp{h}")
                 for h in range(HG)]
        kf01 = pers.tile([128, KEYSP], BF16, tag="kf01")
        kf2 = pers.tile([64, KEYSP], BF16, tag="kf2")
        q01 = pers.tile([128, TOKP], BF16, tag="q01")
        q2 = pers.tile([64, TOKP], BF16, tag="q2")
        vsb = pers.tile([128, NK, HG, DH + 1], BF16, tag="vsb")
        a_sb = [pers.tile([DH, TOKP], BF16, tag=f"a{h}", name=f"a{h}")
                for h in range(HG)]
        gl = pers.tile([128, KO, TOKP], BF16, tag="gl")
        mask_sb = pers.tile([128, 896], BF16, tag="mask")
        cst = pers.tile([128, 20], F32, tag="cst")
        bq01, bq2 = cst[:, 0:1], cst[0:64, 1:2]
        bk01, bk2 = cst[:, 2:3], cst[0:64, 3:4]
        brk01, brk2 = cst[:, 4:5], cst[0:64, 5:6]
        bp4, bfc = cst[:, 6:12], cst[:, 12:18]
        eps_sb = cst[0:1, 19:20]
        rows = pers.tile([1, 1664], F32, tag="rows")
        ones_t = pers.tile([128, DH], BF16, tag="ones_t")
        oner = rows[:, 1152:1664]
        rows16 = pers.tile([1, 1664], BF16, tag="rows16")
        crow = pers.tile([1, 1344], BF16, tag="crow")
        bv_r, brv_r = rows16[:, 0:FG], rows16[:, FG:2 * FG]
        oner16, bmp16_r = rows16[:, 384:896], rows16[:, 896:896 + D]
        cst16 = pers.tile([128, 2], BF16, tag="cst16")
        onec = cst16[:, 0:1]

        # ---- constant / activation input DMAs ----
        pin = lambda t: t.rearrange("(o p) n -> p o n", p=128)
        nc.sync.dma_start(out=h_main[:], in_=pin(hT_d))
        nc.sync.dma_start(out=ctx[:], in_=pin(ctxT_d))
        for h, wpd in enumerate([wp0_d, wp1_d, wp2_d]):
            nc.sync.dma_start(out=wp_sb[h][:], in_=wpd[:])
        nc.sync.dma_start(out=mask_sb[:], in_=mask_d[:])
        nc.sync.dma_start(out=onec, in_=onec_d[:])
        nc.sync.dma_start(out=oner, in_=oner_d[:])
        nc.sync.dma_start(out=oner16, in_=oner16_d[:])
        nc.sync.dma_start(out=crow, in_=crow_d[:])
        nc.sync.dma_start(out=bq01, in_=bq_d[0:128, :])
        nc.sync.dma_start(out=bq2, in_=bq_d[128:FG, :])
        nc.sync.dma_start(out=bk01, in_=bk_d[0:128, :])
        nc.sync.dma_start(out=bk2, in_=bk_d[128:FG, :])
        nc.sync.dma_start(out=brk01, in_=brk_d[0:128, :])
        nc.sync.dma_start(out=brk2, in_=brk_d[128:FG, :])
        nc.sync.dma_start(out=bv_r, in_=bv_d[:])
        nc.sync.dma_start(out=brv_r, in_=brv_d[:])
        nc.sync.dma_start(out=bp4, in_=bp4_d.rearrange("(o p) 1 -> p o", p=128))
        nc.sync.dma_start(out=bfc, in_=bfc_d.rearrange("(o p) 1 -> p o", p=128))
        nc.sync.dma_start(out=bmp16_r, in_=bmp16_d[:])
        nc.vector.memset(eps_sb, EPS)
        nc.vector.memset(ones_t[:], 1.0)
        nc.vector.memset(kf01[:], 0.0)
        nc.vector.memset(kf2[:], 0.0)
        nc.vector.memset(vsb[:], 0.0)

        # ---- LN1 (normalize only; affine folded into wq/wk/wv) ----
        hl = pers.tile([128, KO, TOKP], BF16, tag="ln_out")
        mr1 = _emit_ln(nc, pers, trans, psum, h_main, hl, onec, oner16, eps_sb)

        def load_w(dram_t):
            w = pers.tile([128, KO, FG], BF16, tag="wqkv", bufs=2)
            nc.sync.dma_start(out=w[:], in_=pin(dram_t))
            return w

        # ---- QKV projections (feature-major q/k; token-major v) ----
        def qk_proj(w_sb, b01, b2, dst01, dst2, dst_off, src, src_len,
                    corr=None, mr=None):
            # dst[mi] = w[:, :, mi].T @ src + bias, written at dst col offset
            for dst_t, bias_t, m0, ml in [(dst01, b01, 0, 128),
                                          (dst2, b2, 128, 64)]:
                for bs, bl in [(s, l) for (s, l) in BLKS if s < src_len]:
                    bl = min(bl, src_len - bs)
                    ps = psum.tile([128, 512], F32, tag="sm", bufs=2)
                    for o in range(KO):
                        nc.tensor.matmul(
                            ps[:ml, :bl], w_sb[:, o, ds(m0, ml)],
                            src[:, o, ds(bs, bl)],
                            start=(o == 0),
                            stop=(corr is None and o == KO - 1),
                        )
                    if corr is not None:
                        nc.tensor.matmul(
                            ps[:ml, :bl], crow[0:1, ds(corr + m0, ml)],
                            mr[0:1, ds(bs, bl)], start=False, stop=True,
                        )
                    if dst_off + bs == SC + 1024:
                        # only self token 1024 (key col 1280) is real
                        nc.vector.tensor_scalar_add(
                            dst_t[:ml, 1280:1281], ps[:ml, 0:1],
                            bias_t[:ml, :])
                    else:
                        nc.vector.tensor_scalar_add(
                            dst_t[:ml, ds(dst_off + bs, bl)], ps[:ml, :bl],
                            bias_t[:ml, :])

        def v_proj(w_sb, b_row, src, n_tiles, kt_base, corr=None, mr=None):
            # V[token, feat] = src.T @ w + 1 (x) bias  (token-major output)
            for tt in range(n_tiles):
                ps = psum.tile([128, 512], F32, tag="sm", bufs=2)
                for o in range(KO):
                    nc.tensor.matmul(
                        ps[:, :FG], src[:, o, ts(tt, 128)], w_sb[:, o, :],
                        start=(o == 0), stop=False,
                    )
                if corr is not None:
                    nc.tensor.matmul(ps[:, :FG], mr[0:1, ts(tt, 128)],
                                     crow[0:1, ds(corr, FG)],
                                     start=False, stop=False)
                nc.tensor.matmul(ps[:, :FG], oner16[0:1, 0:128], b_row,
                                 start=False, stop=True)
                kt = kt_base + tt
                rws = 1 if kt == NK - 1 else 128
                nc.vector.tensor_copy(
                    vsb[:rws, kt, :, 0:DH],
                    ps[:rws, 0:FG].rearrange("p (h d) -> p h d", h=HG))

        # q: all padded tokens; k(self): keys 256..1280; kc: keys 0..255
        qk_proj(load_w(wq_d), bq01, bq2, q01, q2, 0, hl, TOKP,
                corr=0, mr=mr1)
        qk_proj(load_w(wk_d), bk01, bk2, kf01, kf2, SC, hl, TOKP,
                corr=FG, mr=mr1)
        qk_proj(load_w(wrk_d), brk01, brk2, kf01, kf2, 0, ctx, SC,
                corr=None, mr=None)
        v_proj(load_w(wrv_d), brv_r, ctx, 2, 0)   # context V -> key tiles 0..1
        v_proj(load_w(wv_d), bv_r, hl, NT, 2, corr=2 * FG, mr=mr1)
        # denominator ones-column (pad rows of the ones column only land in
        # masked or garbage-query positions)
        nc.vector.memset(vsb[:, :, :, DH:DH + 1], 1.0)

        # ---- attention ----
        # per (head, query block): key tiles in groups share one multi-bank
        # scores psum so each exp ACT op covers up to 3x512 (or the whole
        # 128-wide tail) and the ~1.6us fixed ACT cost is amortized.
        kf_of = [(kf01, 0), (kf01, 64), (kf2, 0)]
        q_of = [(q01, 0), (q01, 64), (q2, 0)]
        for h in range(HG):
            kf_t, kf_o = kf_of[h]
            q_t, q_o = q_of[h]
            for qs, ql in BLKS:
                qb0 = qs // 128                      # first 128-query tile
                last_kt = min((qs + ql - 1) // 128 + 2, NK - 1)
                ps_a = psum.tile([128, 512], F32, tag="sm", bufs=2,
                                 name="ps_a")[0:DH + 1]
                gsz = 3 if ql == 512 else 12         # kt per exp group
                for ktg in range(0, last_kt + 1, gsz):
                    kts = range(ktg, min(ktg + gsz, last_kt + 1))
                    ps_s = psum.tile([128, 1536], F32, tag="big3", bufs=2,
                                     name="ps_s")
                    for j, kt in enumerate(kts):
                        nc.tensor.matmul(
                            ps_s[:, ds(j * ql, ql)],
                            kf_t[kf_o:kf_o + DH, ts(kt, 128)],
                            q_t[q_o:q_o + DH, ds(qs, ql)],
                            start=True, stop=True,
                        )
                    expt = trans.tile([128, 1536], BF16, tag="expt", bufs=3)
                    nw = len(kts) * ql
                    nc.scalar.activation(expt[:, :nw], ps_s[:, :nw], AF.Exp,
                                         scale=0.125)
                    for j, kt in enumerate(kts):
                        if kt >= qb0 + 2:  # shifted-triangle mask
                            dlt = kt - qb0 - 2
                            nc.vector.tensor_mul(
                                expt[:, ds(j * ql, ql)],
                                expt[:, ds(j * ql, ql)],
                                mask_sb[:, ds(384 - 128 * dlt, ql)])
                        nc.tensor.matmul(
                            ps_a[:, :ql], vsb[:, kt, h, :],
                            expt[:, ds(j * ql, ql)],
                            start=(kt == 0), stop=(kt == last_kt),
                        )
                # normalize: a = num * (1/den); den sits at partition 64,
                # broadcast to partitions 0..63 via a K=1 PE outer product
                rec = trans.tile([DH + 1, 512], F32, tag="rec", bufs=2)
                r16 = trans.tile([DH + 1, 512], BF16, tag="r16", bufs=2)
                nc.vector.reciprocal(rec[DH:DH + 1, :ql], ps_a[DH:DH + 1, :ql])
                nc.vector.tensor_copy(r16[DH:DH + 1, :ql], rec[DH:DH + 1, :ql])
                ps_r = psum.tile([128, 512], F32, tag="sm", bufs=2,
                                 name="ps_r")[0:DH]
                nc.tensor.matmul(ps_r[:, :ql], ones_t[DH:DH + 1, :],
                                 r16[DH:DH + 1, :ql], start=True, stop=True)
                nc.vector.tensor_copy(rec[0:DH, :ql], ps_r[:, :ql])
                nc.vector.tensor_mul(a_sb[h][:, ds(qs, ql)],
                                     ps_a[0:DH, :ql], rec[0:DH, :ql])

        # ---- attention output projection (row-sharded) + AllReduce ----
        dar_in = dram.tile([KO, 128, TOKP], BF16)
        dar_out = dram.tile([KO, 128, TOKP], BF16)
        for mo in range(KO):
            for bs, bl in BLKS:
                ps = psum.tile([128, 512], F32, tag="sm", bufs=2)
                for h in range(HG):
                    nc.tensor.matmul(ps[:, :bl], wp_sb[h][:, ts(mo, 128)],
                                     a_sb[h][:, ds(bs, bl)],
                                     start=(h == 0), stop=(h == HG - 1))
                armo = trans.tile([128, 512], BF16, tag="armo", bufs=2)
                nc.vector.tensor_scalar_add(armo[:, :bl], ps[:, :bl],
                                            bp4[:, mo:mo + 1])
                nc.sync.dma_start(dar_in[mo, :, ds(bs, bl)], armo[:, :bl])
        if sim_collectives:
            nc.gpsimd.collective_compute(
                "AllReduce", ALU.add, replica_groups=GROUPS,
                ins=[dar_in.opt()], outs=[dar_out.opt()],
            )
        else:
            nc.gpsimd.dma_start(dar_out[:], dar_in[:])
        # residual: h_main <- h_main + allreduced proj output (in place)
        for mo in range(KO):
            for bs, bl in BLKS:
                h2a = trans.tile([128, 512], BF16, tag="h2a", bufs=2)
                nc.sync.dma_start(h2a[:, :bl], dar_out[mo, :, ds(bs, bl)])
                nc.vector.tensor_add(h_main[:, mo, ds(bs, bl)],
                                     h_main[:, mo, ds(bs, bl)], h2a[:, :bl])

        # ---- LN2 + MLP ----
        z0 = pers.tile([128, KO, TOKP], BF16, tag="ln_out")
        mr2 = _emit_ln(nc, pers, trans, psum, h_main, z0, onec, oner16, eps_sb)

        for mg in range(3):  # stream W_fc in thirds of the output dim
            wfc_sb = trans.tile([128, KO, 256], BF16, tag="wbig", bufs=2)
            nc.sync.dma_start(out=wfc_sb[:],
                              in_=pin(wfc_d)[:, :, ds(mg * 256, 256)])
            for mi in range(2):
                mo = mg * 2 + mi
                ps_g = psum.tile([128, 1536], F32, tag="big3", bufs=2)
                for bs, bl in BLKS:
                    for o in range(KO):
                        nc.tensor.matmul(
                            ps_g[:, ds(bs, bl)], wfc_sb[:, o, ts(mi, 128)],
                            z0[:, o, ds(bs, bl)],
                            start=(o == 0), stop=False,
                        )
                    nc.tensor.matmul(
                        ps_g[:, ds(bs, bl)],
                        crow[0:1, ds(3 * FG + mo * 128, 128)],
                        mr2[0:1, ds(bs, bl)], start=False, stop=True,
                    )
                nc.scalar.activation(gl[:, mo, :], ps_g[:, :TOKP], gelu_fn,
                                     bias=bfc[:, mo:mo + 1])

        drs_in = dram.tile([KO, 128, TOKP], BF16)
        drs_out = dram.tile([FG, TOKP], BF16)
        for mg in range(3):
            wmp_sb = trans.tile([128, KO, 256], BF16, tag="wbig", bufs=2)
            nc.sync.dma_start(out=wmp_sb[:],
                              in_=pin(wmp_d)[:, :, ds(mg * 256, 256)])
            for mi in range(2):
                mo = mg * 2 + mi
                ps_mp = psum.tile([128, 1536], F32, tag="big3", bufs=2)
                for bs, bl in BLKS:
                    for o in range(KO):
                        nc.tensor.matmul(
                            ps_mp[:, ds(bs, bl)], wmp_sb[:, o, ts(mi, 128)],
                            gl[:, o, ds(bs, bl)],
                            start=(o == 0), stop=False,
                        )
                    nc.tensor.matmul(
                        ps_mp[:, ds(bs, bl)], bmp16_r[0:1, ts(mo, 128)],
                        oner16[0:1, :bl], start=False, stop=True,
                    )
                mpart = trans.tile([128, TOKP], BF16, tag="mpart", bufs=2)
                # mpart = h_main/4 + (mproj partial + b_mproj/4)
                nc.vector.scalar_tensor_tensor(
                    out=mpart[:], in0=h_main[:, mo, :], scalar=0.25,
                    in1=ps_mp[:, :TOKP], op0=ALU.mult, op1=ALU.add)
                nc.sync.dma_start(drs_in[mo], mpart[:])
        if sim_collectives:
            nc.gpsimd.collective_compute(
                "ReduceScatter", ALU.add, replica_groups=GROUPS,
                ins=[drs_in.opt()], outs=[drs_out.opt()],
            )
        else:
            nc.gpsimd.dma_start(drs_out[0:128, :], drs_in[0, :, :])
            nc.gpsimd.dma_start(drs_out[128:FG, :], drs_in[1, 0:64, :])
        nc.sync.dma_start(out_d[:], drs_out[:])
        if debug_taps:
            for nm, t in [("dbg_hl", hl), ("dbg_q01", q01), ("dbg_kf01", kf01),
                          ("dbg_vsb", vsb), ("dbg_a0", a_sb[0]),
                          ("dbg_h", h_main), ("dbg_z0", z0), ("dbg_gl", gl)]:
                dt_ = t.dtype
                shp = list(t.shape)
                d = nc.dram_tensor(nm, shp, dt_, kind="ExternalOutput")
                nc.sync.dma_start(d[:], t[:])

    nc.compile()
    return nc


_NC_CACHE = None


def _get_program():
    global _NC_CACHE
    if _NC_CACHE is None:
        _NC_CACHE = build_program()
    return _NC_CACHE


def make_in_maps(inputs):
    f = lambda a: np.asarray(a, dtype=np.float32)
    x = f(inputs["x"])
    context_seq = f(inputs["context_seq"])
    sos_h = f(inputs["sos_h"])
    g1, b1 = f(inputs["ln1_g"]), f(inputs["ln1_b"])
    W_attn, b_attn = f(inputs["W_attn"]), f(inputs["b_attn"])
    W_ref, b_ref = f(inputs["W_ref"]), f(inputs["b_ref"])
    W_proj, b_proj = f(inputs["W_proj"]), f(inputs["b_proj"])
    g2, b2 = f(inputs["ln2_g"]), f(inputs["ln2_b"])
    W_fc, b_fc = f(inputs["W_fc"]), f(inputs["b_fc"])
    W_mproj, b_mproj = f(inputs["W_mproj"]), f(inputs["b_mproj"])

    # master causal mask: mask[p, c] = 1 iff p <= c - 384
    cix = np.arange(896)[None, :]
    pix = np.arange(128)[:, None]
    mask = (pix <= cix - 384).astype(np.float32)
    onec = np.ones((128, 1), np.float32)
    oner = np.ones((1, 512), np.float32)

    wfc_g = W_fc * g2[:, None]
    bfc_full = b2 @ W_fc + b_fc

    in_maps = []
    for core in range(N_CORES):
        b, g = core // TP, core % TP
        h = np.concatenate([sos_h[None, :], x[b]], axis=0)  # [1025, 768]
        hT = np.zeros((D, TOKP), ml_dtypes.bfloat16)
        hT[:, :TOK] = h.T.astype(ml_dtypes.bfloat16)
        qsl = slice(FG * g, FG * (g + 1))
        ksl = slice(D + FG * g, D + FG * (g + 1))
        vsl = slice(2 * D + FG * g, 2 * D + FG * (g + 1))
        rks = slice(FG * g, FG * (g + 1))
        rvs = slice(D + FG * g, D + FG * (g + 1))
        mcols = slice(D * g, D * (g + 1))    # W_fc column slice (768 per core)
        wp_slab = W_proj[FG * g:FG * (g + 1), :]
        bf = ml_dtypes.bfloat16
        in_maps.append({
            "hT": hT,
            "ctxT": np.ascontiguousarray(context_seq[b].T).astype(bf),
            "wq": np.ascontiguousarray(W_attn[:, qsl] * g1[:, None]).astype(bf),
            "wk": np.ascontiguousarray(W_attn[:, ksl] * g1[:, None]).astype(bf),
            "wv": np.ascontiguousarray(W_attn[:, vsl] * g1[:, None]).astype(bf),
            "wrk": np.ascontiguousarray(W_ref[:, rks]).astype(bf),
            "wrv": np.ascontiguousarray(W_ref[:, rvs]).astype(bf),
            "bq": (b1 @ W_attn[:, qsl] + b_attn[qsl]).reshape(FG, 1),
            "bk": (b1 @ W_attn[:, ksl] + b_attn[ksl]).reshape(FG, 1),
            "brk": b_ref[rks].reshape(FG, 1),
            "bv_r": (b1 @ W_attn[:, vsl] + b_attn[vsl]).reshape(1, FG).astype(bf),
            "brv_r": b_ref[rvs].reshape(1, FG).astype(bf),
            "wp0": np.ascontiguousarray(wp_slab[0:64, :]).astype(bf),
            "wp1": np.ascontiguousarray(wp_slab[64:128, :]).astype(bf),
            "wp2": np.ascontiguousarray(wp_slab[128:192, :]).astype(bf),
            "bp4": (b_proj / TP).reshape(D, 1),
            "wfc": np.ascontiguousarray(wfc_g[:, mcols]).astype(bf),
            "bfc": bfc_full[mcols].reshape(D, 1),
            "wmp": np.ascontiguousarray(W_mproj[mcols, :]).astype(bf),
            "bmp16_r": (b_mproj / TP).reshape(1, D).astype(bf),
            "mask": mask.astype(bf),
            "crow": np.concatenate([
                -(W_attn[:, qsl] * g1[:, None]).sum(0),
                -(W_attn[:, ksl] * g1[:, None]).sum(0),
                -(W_attn[:, vsl] * g1[:, None]).sum(0),
                -wfc_g[:, mcols].sum(0),
            ]).reshape(1, 1344).astype(bf),
            "onec": onec.astype(bf),
            "oner": oner,
            "oner16": oner.astype(bf),
        })
    return in_maps


def assemble_output(results, B=2):
    out = np.empty((B, S, D), np.float32)
    for b in range(B):
        parts = [np.asarray(results[TP * b + g]["out"], np.float32)
                 for g in range(TP)]
        full = np.concatenate(parts, axis=0)  # [768, 1152]
        out[b] = full[:, 1:TOK].T
    return out


def kernel(**inputs):
    nc = _get_program()
    in_maps = make_in_maps(inputs)
    res = run_bass_kernel_spmd(nc, in_maps, list(range(N_CORES)))
    return assemble_output(res.results, B=np.asarray(inputs["x"]).shape[0])


if __name__ == "__main__":
    import reference
    ins = reference.setup_inputs()
    ins = {k: np.asarray(v) for k, v in ins.items()}
    got = kernel(**ins)
    exp = np.asarray(reference.reference(**ins))
    err = np.abs(got - exp).max() / np.abs(exp).max()
    print("max abs err:", np.abs(got - exp).max(), "rel:", err)

